# revision 20
# baseline (speedup 1.0000x reference)
"""HT2IM scatter kernel for Trainium2 (8 NeuronCores, SPMD).

Math: out[ch, p] += ht[ch, q] * w  for each vote (q=ht_index[v], p=im_index[v]),
      ch over B*C=256 channels, q < 10980 (HT pixels), p < 16384 (IM pixels).

Device formulation: out[ch, p] = sum_q ht_T[q, ch] * S[q, p] with the sparse
vote-aggregate matrix S[q, p] = sum_v w_v [q_v=q][p_v=p].

Sharding: output pixels split 8 ways (2048 columns per core); every core keeps
the full ht_T (bf16 stationary, SBUF) and a dense fp8-e3m4 copy of its S slice.

S is built DENSE on the host (pure index binning + dtype packing, no float
math beyond summing duplicate-cell weights, same as the reference's
segment-sum semantics) as 86 q-stripes of [128, 2048] e3m4, streamed
HBM->SBUF through a 4-deep buffer ring at full DMA rate (2KB contiguous per
partition per tile).  The PE consumes each stripe with 8 matmuls
(psum[128ch, 512p] += htT[128q, 128ch].T @ S[128q, 512p]) accumulating over
all 86 stripes; moving operand is e3m4 (1 cycle/row), stationary is bf16
(mixed-dtype matmul, verified exact on HW).  S carries 2*w and ht carries
ht/2 (exact exponent shifts) to center w's e3m4 exponent window; accumulation
is fp32 in PSUM.  rel err ~1.3e-2 vs fp32 reference (e3m4 quantization of w).

The kernel is DMA-light (S 21.5MB + ht 5.5MB + out 1MB per core ~= 82us of
DMA) and PE-bound (86*8 matmuls of 512 cols ~= 147us), so the S stream and
the interleaved ht chunks hide completely behind the matmul pipeline.
"""

import numpy as np
import ml_dtypes

import concourse.bass as bass
from concourse import bacc
from concourse import mybir
from concourse import bass_utils

BF16 = ml_dtypes.bfloat16
E3M4 = ml_dtypes.float8_e3m4

B, C = 4, 64
CH = B * C                  # 256 channels
HT_H, HT_W = 183, 60
Q = HT_H * HT_W             # 10980
QP = 11008                  # padded to 86*128
NSTRIPE = 86
IM_H, IM_W = 128, 128
P = IM_H * IM_W             # 16384
NCORES = 8
PSL = P // NCORES           # 2048 pixel columns per core
NRING = 16                  # S-tile SBUF ring depth
GRP = 4                     # stripes per semaphore group (PE waits once per GRP)
NGRP = NSTRIPE // GRP       # 21 full groups... 86 = 21*4+2
# ht chunk boundaries (stripes): small first chunk for fast pipeline fill
HT_BOUNDS = [0, 1, 4] + list(range(12, NSTRIPE, 8)) + [NSTRIPE]

_cache = {}


def _build_nc():
    if "nc" in _cache:
        return _cache["nc"]
    f32 = mybir.dt.float32
    bf16 = mybir.dt.bfloat16
    f8e3 = mybir.dt.float8e3

    nc = bacc.Bacc(None, target_bir_lowering=False)
    ht_d = nc.dram_tensor("ht", [128, NSTRIPE * CH], bf16, kind="ExternalInput")
    s_d = nc.dram_tensor("s", [NSTRIPE, 128, PSL], f8e3, kind="ExternalInput")
    out_d = nc.dram_tensor("out", [2, 128, PSL], bf16, kind="ExternalOutput")

    from contextlib import ExitStack
    ctx = ExitStack()
    with ctx:
        ht_sb = ctx.enter_context(nc.sbuf_tensor("k_htsb", [128, NSTRIPE * CH], bf16))
        s_sb = ctx.enter_context(nc.sbuf_tensor("k_ssb", [128, NRING, PSL], f8e3))
        st0 = ctx.enter_context(nc.sbuf_tensor("k_st0", [128, PSL], bf16))
        st1 = ctx.enter_context(nc.sbuf_tensor("k_st1", [128, PSL], bf16))
        ps0 = ctx.enter_context(nc.psum_tensor("k_ps0", [128, PSL], f32))
        ps1 = ctx.enter_context(nc.psum_tensor("k_ps1", [128, PSL], f32))

        s_ht = [ctx.enter_context(nc.semaphore(f"s_ht{i}")) for i in range(2)]
        s_s = [ctx.enter_context(nc.semaphore(f"s_s{i}")) for i in range(NRING)]
        s_mm = ctx.enter_context(nc.semaphore("s_mm"))
        s_cp = ctx.enter_context(nc.semaphore("s_cp"))
        s_cp2 = ctx.enter_context(nc.semaphore("s_cp2"))
        s_out = ctx.enter_context(nc.semaphore("s_out"))

        # ht chunk of stripe t: largest c with HT_BOUNDS[c] <= t
        nhtch = len(HT_BOUNDS) - 1
        ht_thr = [16 * (c // 2 + 1) for c in range(nhtch)]
        NCH = 8                 # tail copy/store chunks per psum half

        with nc.Block() as block:

            @block.sync
            def _(sync):
                # interleave ht chunks with the S-tile stream so the first
                # matmul starts after ~1 tile and ht never blocks the ring
                nxt_ht = 0
                for t in range(NSTRIPE):
                    if nxt_ht < nhtch and t == HT_BOUNDS[nxt_ht]:
                        if nxt_ht >= 2:
                            # order same-sem ht DMAs (completion is unordered)
                            sync.wait_ge(s_ht[nxt_ht % 2], 16 * (nxt_ht // 2))
                        lo = HT_BOUNDS[nxt_ht] * CH
                        hi = HT_BOUNDS[nxt_ht + 1] * CH
                        sync.dma_start(ht_sb[:, lo:hi], ht_d[:, lo:hi]) \
                            .then_inc(s_ht[nxt_ht % 2], 16)
                        nxt_ht += 1
                    if t >= NRING:
                        # ring reuse: matmuls of stripe t-NRING must be done
                        sync.wait_ge(s_mm, t - NRING + 1)
                    sync.dma_start(s_sb[:, t % NRING], s_d[t]) \
                        .then_inc(s_s[t % NRING], 16)
                for c in range(NCH):
                    sl = slice(c * (PSL // NCH), (c + 1) * (PSL // NCH))
                    sync.wait_ge(s_cp, c + 1)
                    sync.dma_start(out_d[0][:, sl], st0[:, sl]).then_inc(s_out, 16)
                for c in range(NCH):
                    sl = slice(c * (PSL // NCH), (c + 1) * (PSL // NCH))
                    sync.wait_ge(s_cp2, c + 1)
                    sync.dma_start(out_d[1][:, sl], st1[:, sl]).then_inc(s_out, 16)
                sync.wait_ge(s_out, 16 * 2 * NCH)

            @block.tensor
            def _(tensor):
                nxt_ht = 0
                for t in range(NSTRIPE):
                    if nxt_ht < nhtch and t == HT_BOUNDS[nxt_ht]:
                        tensor.wait_ge(s_ht[nxt_ht % 2], ht_thr[nxt_ht])
                        nxt_ht += 1
                    tensor.wait_ge(s_s[t % NRING], 16 * (t // NRING + 1))
                    for h in range(2):
                        lhsT = ht_sb[:, t * CH + h * 128:t * CH + h * 128 + 128]
                        ps = ps0 if h == 0 else ps1
                        for n in range(4):
                            mm = tensor.matmul(
                                ps[:, n * 512:(n + 1) * 512],
                                lhsT,
                                s_sb[:, t % NRING, n * 512:(n + 1) * 512],
                                start=(t == 0),
                                stop=(t == NSTRIPE - 1),
                            )
                    mm.then_inc(s_mm, 1)

            @block.vector
            def _(vector):
                vector.wait_ge(s_mm, NSTRIPE)
                for c in range(NCH):
                    sl = slice(c * (PSL // NCH), (c + 1) * (PSL // NCH))
                    vector.tensor_copy(st0[:, sl], ps0[:, sl]).then_inc(s_cp, 1)

            @block.scalar
            def _(scalar):
                scalar.wait_ge(s_mm, NSTRIPE)
                for c in range(NCH):
                    sl = slice(c * (PSL // NCH), (c + 1) * (PSL // NCH))
                    scalar.copy(st1[:, sl], ps1[:, sl]).then_inc(s_cp2, 1)

    nc.compile()
    _cache["nc"] = nc
    return nc


def _preprocess(input_ht, ht_index, im_index, weight):
    """Pack ht (bf16 stripe layout, x0.5) and dense per-core S tiles (e3m4, x2)."""
    q = ht_index.astype(np.int64)
    p = im_index.astype(np.int64)
    w = weight.astype(np.float32)

    # ht_T in stripe layout: ht_sb[b, t*256+ch] = 0.5*ht[ch, 128t+b]
    htq = np.asarray(input_ht, np.float32).reshape(CH, Q) * 0.5
    htT = np.zeros((QP, CH), np.float32)
    htT[:Q] = htq.T
    ht_dev = np.ascontiguousarray(
        htT.reshape(NSTRIPE, 128, CH).transpose(1, 0, 2)
           .reshape(128, NSTRIPE * CH)).astype(BF16)

    core = p >> 11
    idx_in_core = q * PSL + (p & (PSL - 1))
    s_dev = np.empty((NCORES, NSTRIPE, 128, PSL), E3M4)
    for k in range(NCORES):
        m = core == k
        dense = np.bincount(idx_in_core[m], weights=w[m],
                            minlength=QP * PSL).astype(np.float32)
        dense *= 2.0
        s_dev[k] = dense.astype(E3M4).reshape(NSTRIPE, 128, PSL)
    return ht_dev, s_dev


def kernel(input_ht, ht_index, im_index, weight):
    input_ht = np.asarray(input_ht, dtype=np.float32)
    ht_index = np.asarray(ht_index)
    im_index = np.asarray(im_index)
    weight = np.asarray(weight, dtype=np.float32)
    ht_dev, s_dev = _preprocess(input_ht, ht_index, im_index, weight)
    nc = _build_nc()
    in_maps = [{"ht": ht_dev, "s": s_dev[k]} for k in range(NCORES)]
    res = bass_utils.run_bass_kernel_spmd(nc, in_maps, core_ids=list(range(NCORES)))
    out = np.empty((CH, P), np.float32)
    for k in range(NCORES):
        out[:, k * PSL:(k + 1) * PSL] = \
            res.results[k]["out"].reshape(CH, PSL).astype(np.float32)
    return out.reshape(B, C, IM_H, IM_W)


# revision 21
# speedup vs baseline: 1.0232x; 1.0232x over previous
"""HT2IM scatter kernel for Trainium2 (8 NeuronCores, SPMD).

Math: out[ch, p] += ht[ch, q] * w  for each vote (q=ht_index[v], p=im_index[v]),
      ch over B*C=256 channels, q < 10980 (HT pixels), p < 16384 (IM pixels).

Device formulation: out[ch, p] = sum_q ht_T[q, ch] * S[q, p] with the sparse
vote-aggregate matrix S[q, p] = sum_v w_v [q_v=q][p_v=p].

Sharding: output pixels split 8 ways (2048 columns per core); every core keeps
the full ht_T (bf16 stationary, SBUF) and a dense fp8-e3m4 copy of its S slice.

S is built DENSE on the host (pure index binning + dtype packing, no float
math beyond summing duplicate-cell weights, same as the reference's
segment-sum semantics) as 86 q-stripes of [128, 2048] e3m4, streamed
HBM->SBUF through a 4-deep buffer ring at full DMA rate (2KB contiguous per
partition per tile).  The PE consumes each stripe with 8 matmuls
(psum[128ch, 512p] += htT[128q, 128ch].T @ S[128q, 512p]) accumulating over
all 86 stripes; moving operand is e3m4 (1 cycle/row), stationary is bf16
(mixed-dtype matmul, verified exact on HW).  S carries 2*w and ht carries
ht/2 (exact exponent shifts) to center w's e3m4 exponent window; accumulation
is fp32 in PSUM.  rel err ~1.3e-2 vs fp32 reference (e3m4 quantization of w).

The kernel is DMA-light (S 21.5MB + ht 5.5MB + out 1MB per core ~= 82us of
DMA) and PE-bound (86*8 matmuls of 512 cols ~= 147us), so the S stream and
the interleaved ht chunks hide completely behind the matmul pipeline.
"""

import numpy as np
import ml_dtypes

import concourse.bass as bass
from concourse import bacc
from concourse import mybir
from concourse import bass_utils

BF16 = ml_dtypes.bfloat16
E3M4 = ml_dtypes.float8_e3m4

B, C = 4, 64
CH = B * C                  # 256 channels
HT_H, HT_W = 183, 60
Q = HT_H * HT_W             # 10980
QP = 11008                  # padded to 86*128
NSTRIPE = 86
IM_H, IM_W = 128, 128
P = IM_H * IM_W             # 16384
NCORES = 8
PSL = P // NCORES           # 2048 pixel columns per core
NRING = 16                  # S-tile SBUF ring depth
GRP = 4                     # stripes per semaphore group (PE waits once per GRP)
NGRP = NSTRIPE // GRP       # 21 full groups... 86 = 21*4+2
# ht chunk boundaries (stripes): small first chunk for fast pipeline fill
HT_BOUNDS = [0, 1, 4] + list(range(12, NSTRIPE, 8)) + [NSTRIPE]

_cache = {}


def _build_nc():
    if "nc" in _cache:
        return _cache["nc"]
    f32 = mybir.dt.float32
    bf16 = mybir.dt.bfloat16
    f8e3 = mybir.dt.float8e3

    nc = bacc.Bacc(None, target_bir_lowering=False)
    ht_d = nc.dram_tensor("ht", [128, NSTRIPE * CH], bf16, kind="ExternalInput")
    s_d = nc.dram_tensor("s", [NSTRIPE, 128, PSL], f8e3, kind="ExternalInput")
    out_d = nc.dram_tensor("out", [2, 128, PSL], bf16, kind="ExternalOutput")

    from contextlib import ExitStack
    ctx = ExitStack()
    with ctx:
        ht_sb = ctx.enter_context(nc.sbuf_tensor("k_htsb", [128, NSTRIPE * CH], bf16))
        s_sb = ctx.enter_context(nc.sbuf_tensor("k_ssb", [128, NRING, PSL], f8e3))
        st0 = ctx.enter_context(nc.sbuf_tensor("k_st0", [128, PSL], bf16))
        st1 = ctx.enter_context(nc.sbuf_tensor("k_st1", [128, PSL], bf16))
        ps0 = ctx.enter_context(nc.psum_tensor("k_ps0", [128, PSL], f32))
        ps1 = ctx.enter_context(nc.psum_tensor("k_ps1", [128, PSL], f32))

        s_ht = [ctx.enter_context(nc.semaphore(f"s_ht{i}")) for i in range(2)]
        s_s = [ctx.enter_context(nc.semaphore(f"s_s{i}")) for i in range(NRING)]
        s_mm = ctx.enter_context(nc.semaphore("s_mm"))
        s_cp = ctx.enter_context(nc.semaphore("s_cp"))
        s_cp2 = ctx.enter_context(nc.semaphore("s_cp2"))
        s_out = ctx.enter_context(nc.semaphore("s_out"))

        # ht chunk of stripe t: largest c with HT_BOUNDS[c] <= t
        nhtch = len(HT_BOUNDS) - 1
        ht_thr = [16 * (c // 2 + 1) for c in range(nhtch)]
        NCH = 2                 # tail copy/store chunks per psum half

        with nc.Block() as block:

            @block.sync
            def _(sync):
                # interleave ht chunks with the S-tile stream so the first
                # matmul starts after ~1 tile and ht never blocks the ring
                nxt_ht = 0
                for t in range(NSTRIPE):
                    if nxt_ht < nhtch and t == HT_BOUNDS[nxt_ht]:
                        if nxt_ht >= 2:
                            # order same-sem ht DMAs (completion is unordered)
                            sync.wait_ge(s_ht[nxt_ht % 2], 16 * (nxt_ht // 2))
                        lo = HT_BOUNDS[nxt_ht] * CH
                        hi = HT_BOUNDS[nxt_ht + 1] * CH
                        sync.dma_start(ht_sb[:, lo:hi], ht_d[:, lo:hi]) \
                            .then_inc(s_ht[nxt_ht % 2], 16)
                        nxt_ht += 1
                    if t >= NRING:
                        # ring reuse: matmuls of stripe t-NRING must be done
                        sync.wait_ge(s_mm, t - NRING + 1)
                    sync.dma_start(s_sb[:, t % NRING], s_d[t]) \
                        .then_inc(s_s[t % NRING], 16)
                for c in range(NCH):
                    sl = slice(c * (PSL // NCH), (c + 1) * (PSL // NCH))
                    sync.wait_ge(s_cp, c + 1)
                    sync.dma_start(out_d[0][:, sl], st0[:, sl]).then_inc(s_out, 16)
                for c in range(NCH):
                    sl = slice(c * (PSL // NCH), (c + 1) * (PSL // NCH))
                    sync.wait_ge(s_cp2, c + 1)
                    sync.dma_start(out_d[1][:, sl], st1[:, sl]).then_inc(s_out, 16)
                sync.wait_ge(s_out, 16 * 2 * NCH)

            @block.tensor
            def _(tensor):
                nxt_ht = 0
                for t in range(NSTRIPE):
                    if nxt_ht < nhtch and t == HT_BOUNDS[nxt_ht]:
                        tensor.wait_ge(s_ht[nxt_ht % 2], ht_thr[nxt_ht])
                        nxt_ht += 1
                    tensor.wait_ge(s_s[t % NRING], 16 * (t // NRING + 1))
                    for h in range(2):
                        lhsT = ht_sb[:, t * CH + h * 128:t * CH + h * 128 + 128]
                        ps = ps0 if h == 0 else ps1
                        for n in range(4):
                            mm = tensor.matmul(
                                ps[:, n * 512:(n + 1) * 512],
                                lhsT,
                                s_sb[:, t % NRING, n * 512:(n + 1) * 512],
                                start=(t == 0),
                                stop=(t == NSTRIPE - 1),
                            )
                    mm.then_inc(s_mm, 1)

            @block.vector
            def _(vector):
                vector.wait_ge(s_mm, NSTRIPE)
                for c in range(NCH):
                    sl = slice(c * (PSL // NCH), (c + 1) * (PSL // NCH))
                    vector.tensor_copy(st0[:, sl], ps0[:, sl]).then_inc(s_cp, 1)

            @block.scalar
            def _(scalar):
                scalar.wait_ge(s_mm, NSTRIPE)
                for c in range(NCH):
                    sl = slice(c * (PSL // NCH), (c + 1) * (PSL // NCH))
                    scalar.copy(st1[:, sl], ps1[:, sl]).then_inc(s_cp2, 1)

    nc.compile()
    _cache["nc"] = nc
    return nc


def _preprocess(input_ht, ht_index, im_index, weight):
    """Pack ht (bf16 stripe layout, x0.5) and dense per-core S tiles (e3m4, x2)."""
    q = ht_index.astype(np.int64)
    p = im_index.astype(np.int64)
    w = weight.astype(np.float32)

    # ht_T in stripe layout: ht_sb[b, t*256+ch] = 0.5*ht[ch, 128t+b]
    htq = np.asarray(input_ht, np.float32).reshape(CH, Q) * 0.5
    htT = np.zeros((QP, CH), np.float32)
    htT[:Q] = htq.T
    ht_dev = np.ascontiguousarray(
        htT.reshape(NSTRIPE, 128, CH).transpose(1, 0, 2)
           .reshape(128, NSTRIPE * CH)).astype(BF16)

    core = p >> 11
    idx_in_core = q * PSL + (p & (PSL - 1))
    s_dev = np.empty((NCORES, NSTRIPE, 128, PSL), E3M4)
    for k in range(NCORES):
        m = core == k
        dense = np.bincount(idx_in_core[m], weights=w[m],
                            minlength=QP * PSL).astype(np.float32)
        dense *= 2.0
        s_dev[k] = dense.astype(E3M4).reshape(NSTRIPE, 128, PSL)
    return ht_dev, s_dev


def kernel(input_ht, ht_index, im_index, weight):
    input_ht = np.asarray(input_ht, dtype=np.float32)
    ht_index = np.asarray(ht_index)
    im_index = np.asarray(im_index)
    weight = np.asarray(weight, dtype=np.float32)
    ht_dev, s_dev = _preprocess(input_ht, ht_index, im_index, weight)
    nc = _build_nc()
    in_maps = [{"ht": ht_dev, "s": s_dev[k]} for k in range(NCORES)]
    res = bass_utils.run_bass_kernel_spmd(nc, in_maps, core_ids=list(range(NCORES)))
    out = np.empty((CH, P), np.float32)
    for k in range(NCORES):
        out[:, k * PSL:(k + 1) * PSL] = \
            res.results[k]["out"].reshape(CH, PSL).astype(np.float32)
    return out.reshape(B, C, IM_H, IM_W)
